# revision 9
# baseline (speedup 1.0000x reference)
"""Conv2d 3x3 (pad 1, stride 1) + bias on 8 Trainium2 cores.

Problem: x [32,128,56,56] f32, weights [256,128,3,3] f32, bias [256] f32
         -> out [32,256,56,56] f32.

Strategy
--------
Data-parallel over batch: each of the 8 cores owns 4 images.

Per core, implicit GEMM with a width/height-padded image layout:
  - Host pads each image to [128, 58, 58] (zeros ring) and flattens to
    [128, 3364]; one contiguous DMA per image brings it to SBUF.
  - Output is computed in a "padded-row" layout [256, 56*58]: output row
    oh occupies columns [oh*58, oh*58+56); the 2 trailing columns per row
    are junk. In this layout every tap (kh, kw) of the 3x3 kernel is a
    CONSTANT offset kh*58+kw into the flat padded input, so one matmul
    covers 8 output rows at once (N = 8*58 = 464 <= 512 PSUM bank).
  - Weights are host-transposed to lhsT layout [Cin=128, (kh,kw,Cout)]
    so each tap/Cout-half slice is a contiguous [128,128] stationary tile.
  - 9 taps accumulate into one PSUM tile (start/stop flags); Cout=256 is
    split into 2 halves of 128 partitions.
  - float32r matmuls: full PE rate (1 cycle/row) at N>=256 with ~fp32
    accuracy (fp32 split into bf16 hi/lo inside the PE).
  - Bias is fused into the PSUM->SBUF copy via DVE tensor_scalar_add with
    a per-partition [128,1] scalar.
  - Host strips the junk columns: out[..., :56] of [56,58].

Built on bacc.Bacc (not raw bass.Bass): walrus engine-instruction structs
hold at most ONE sync wait, and Bacc's compile() runs the
move_matmul_waits_to_ldweights / generate_event_semaphores passes that
split excess waits into EventSemaphore instructions.
"""

import numpy as np

import concourse.bacc as bacc
import concourse.mybir as mybir
import concourse.tile as tile
from concourse.bass_utils import run_bass_kernel_spmd

B, CIN, H, W = 32, 128, 56, 56
COUT = 256
NCORES = 8
BLOC = B // NCORES  # images per core
HP, WP = H + 2, W + 2  # 58, 58
NPIX = HP * WP  # 3364
OUTW = H * WP  # 3248 output columns in padded-row layout
TILE_N = 8 * WP  # 464: 8 output rows per PSUM tile
NTILES = 7  # 7 tiles x 8 rows = 56 rows
# Last tile is trimmed by 2 so the bottom-right tap never reads past the
# padded image (it would only have fed the final 2 junk columns).
LAST_N = TILE_N - 2

_nc_cache = None


def _build():
    f32 = mybir.dt.float32
    f32r = mybir.dt.float32r
    nc = bacc.Bacc("TRN2", target_bir_lowering=False)
    x_d = nc.dram_tensor("xp", [BLOC, CIN, NPIX], f32r, kind="ExternalInput")
    w_d = nc.dram_tensor("wT", [CIN, 9 * COUT], f32r, kind="ExternalInput")
    b_d = nc.dram_tensor("bias2", [128, 2], f32, kind="ExternalInput")
    o_d = nc.dram_tensor("out", [BLOC, COUT, OUTW], f32, kind="ExternalOutput")

    with tile.TileContext(nc) as tc:
        with (
            tc.tile_pool(name="wpool", bufs=1) as wpool,
            tc.tile_pool(name="xpool", bufs=2) as xpool,
            tc.tile_pool(name="opool", bufs=4) as opool,
            tc.tile_pool(name="psum", bufs=4, space="PSUM") as psum,
        ):
            wsb = wpool.tile([CIN, 9 * COUT], f32r)
            nc.sync.dma_start(wsb[:], w_d[:])
            bsb = wpool.tile([128, 2], f32)
            nc.sync.dma_start(bsb[:], b_d[:])

            for b in range(BLOC):
                xp = xpool.tile([CIN, NPIX], f32r)
                nc.sync.dma_start(xp[:], x_d[b])
                for h in range(2):
                    for t in range(NTILES):
                        n = TILE_N if t < NTILES - 1 else LAST_N
                        pt = psum.tile([128, TILE_N], f32)
                        for tap in range(9):
                            kh, kw = divmod(tap, 3)
                            off = t * TILE_N + kh * WP + kw
                            c0 = tap * COUT + h * 128
                            nc.tensor.matmul(
                                pt[:, :n],
                                lhsT=wsb[:, c0 : c0 + 128],
                                rhs=xp[:, off : off + n],
                                start=(tap == 0),
                                stop=(tap == 8),
                            )
                        ot = opool.tile([128, TILE_N], f32)
                        nc.vector.tensor_scalar_add(
                            ot[:, :n], pt[:, :n], bsb[:, h : h + 1]
                        )
                        nc.sync.dma_start(
                            o_d[b, h * 128 : (h + 1) * 128, t * TILE_N : t * TILE_N + n],
                            ot[:, :n],
                        )
    nc.compile()
    return nc


def _get_nc():
    global _nc_cache
    if _nc_cache is None:
        _nc_cache = _build()
    return _nc_cache


def _prep_inputs(x, weights, bias):
    x = np.ascontiguousarray(np.asarray(x, dtype=np.float32))
    weights = np.ascontiguousarray(np.asarray(weights, dtype=np.float32))
    bias = np.ascontiguousarray(np.asarray(bias, dtype=np.float32))

    xp = np.pad(x, ((0, 0), (0, 0), (1, 1), (1, 1))).reshape(B, CIN, NPIX)
    # [Cout,Cin,3,3] -> [Cin, (kh kw Cout)] so lhsT tap slices are contiguous
    wT = np.ascontiguousarray(weights.transpose(1, 2, 3, 0)).reshape(CIN, 9 * COUT)
    b2 = np.ascontiguousarray(bias.reshape(2, 128).T)  # b2[p, h] = bias[h*128+p]

    return [
        {
            "xp": np.ascontiguousarray(xp[i * BLOC : (i + 1) * BLOC]),
            "wT": wT,
            "bias2": b2,
        }
        for i in range(NCORES)
    ]


def _run(inputs, trace=False):
    in_maps = _prep_inputs(inputs["x"], inputs["weights"], inputs["bias"])
    res = run_bass_kernel_spmd(
        _get_nc(), in_maps, core_ids=list(range(NCORES)), trace=trace
    )
    out = np.concatenate([r["out"] for r in res.results], axis=0)
    out = out.reshape(B, COUT, H, WP)[:, :, :, :W]
    return np.ascontiguousarray(out), res


def kernel(x, weights, bias):
    out, _ = _run({"x": x, "weights": weights, "bias": bias})
    return out


# revision 13
# speedup vs baseline: 1.0597x; 1.0597x over previous
"""Conv2d 3x3 (pad 1, stride 1) + bias on 8 Trainium2 cores.

Problem: x [32,128,56,56] f32, weights [256,128,3,3] f32, bias [256] f32
         -> out [32,256,56,56] f32.

Strategy
--------
Data-parallel over batch: each of the 8 cores owns 4 images.

Per core, implicit GEMM with a width/height-padded image layout:
  - Host pads each image to [128, 58, 58] (zeros ring) and flattens to
    [128, 3364]; one contiguous DMA per image brings it to SBUF.
  - Output is computed in a "padded-row" layout [256, 56*58]: output row
    oh occupies columns [oh*58, oh*58+56); the 2 trailing columns per row
    are junk. In this layout every tap (kh, kw) of the 3x3 kernel is a
    CONSTANT offset kh*58+kw into the flat padded input, so one matmul
    covers 8 output rows at once (N = 8*58 = 464 <= 512 PSUM bank).
  - Weights are host-transposed to lhsT layout [Cin=128, (kh,kw,Cout)]
    so each tap/Cout-half slice is a contiguous [128,128] stationary tile.
  - 9 taps accumulate into one PSUM tile (start/stop flags); Cout=256 is
    split into 2 halves of 128 partitions.
  - float32r matmuls: full PE rate (1 cycle/row) at N>=256 with ~fp32
    accuracy (fp32 split into bf16 hi/lo inside the PE).
  - Bias is fused into the PSUM->SBUF copy via DVE tensor_scalar_add with
    a per-partition [128,1] scalar.
  - Host strips the junk columns: out[..., :56] of [56,58].

Built on bacc.Bacc (not raw bass.Bass): walrus engine-instruction structs
hold at most ONE sync wait, and Bacc's compile() runs the
move_matmul_waits_to_ldweights / generate_event_semaphores passes that
split excess waits into EventSemaphore instructions.
"""

import numpy as np

import concourse.bacc as bacc
import concourse.mybir as mybir
import concourse.tile as tile
from concourse.bass_utils import run_bass_kernel_spmd

B, CIN, H, W = 32, 128, 56, 56
COUT = 256
NCORES = 8
BLOC = B // NCORES  # images per core
HP, WP = H + 2, W + 2  # 58, 58
NPIX = HP * WP  # 3364
OUTW = H * WP  # 3248 output columns in padded-row layout
TILE_N = 8 * WP  # 464: 8 output rows per PSUM tile
NTILES = 7  # 7 tiles x 8 rows = 56 rows
# Last tile is trimmed by 2 so the bottom-right tap never reads past the
# padded image (it would only have fed the final 2 junk columns).
LAST_N = TILE_N - 2
# Image DMAs land in 4 chunks so early matmul groups start sooner.
XCHUNKS = 4
XCHUNK = NPIX // XCHUNKS  # 841

_nc_cache = None


def _build():
    f32 = mybir.dt.float32
    f32r = mybir.dt.float32r
    nc = bacc.Bacc("TRN2", target_bir_lowering=False)
    x_d = nc.dram_tensor("xp", [BLOC, CIN, NPIX], f32r, kind="ExternalInput")
    w_d = nc.dram_tensor("wT", [CIN, 9 * COUT], f32r, kind="ExternalInput")
    b_d = nc.dram_tensor("bias2", [128, 2], f32, kind="ExternalInput")
    o_d = nc.dram_tensor("out", [BLOC, COUT, OUTW], f32, kind="ExternalOutput")

    with tile.TileContext(nc) as tc:
        with (
            tc.tile_pool(name="wpool", bufs=1) as wpool,
            tc.tile_pool(name="xpool", bufs=2) as xpool,
            tc.tile_pool(name="opool", bufs=4) as opool,
            tc.tile_pool(name="psum", bufs=4, space="PSUM") as psum,
        ):
            # Weight layout [Cin, (half, tap, co)]: half-0's taps are the
            # first 1152 cols, so the first matmul group only gates on the
            # first of the two weight DMAs. Image DMAs are chunked so
            # early matmul groups start as soon as their rows have landed
            # (Tile tracks dependencies per address range).
            wsb = wpool.tile([CIN, 9 * COUT], f32r)
            nc.sync.dma_start(wsb[:, : 9 * 128], w_d[:, : 9 * 128])
            bsb = wpool.tile([128, 2], f32)

            for b in range(BLOC):
                xp = xpool.tile([CIN, NPIX], f32r, tag="xp")
                for c in range(XCHUNKS):
                    lo = c * XCHUNK
                    hi = min(NPIX, lo + XCHUNK)
                    nc.sync.dma_start(xp[:, lo:hi], x_d[b, :, lo:hi])
                if b == 0:
                    nc.sync.dma_start(wsb[:, 9 * 128 :], w_d[:, 9 * 128 :])
                    nc.sync.dma_start(bsb[:], b_d[:])
                for h in range(2):
                    for t in range(NTILES):
                        n = TILE_N if t < NTILES - 1 else LAST_N
                        pt = psum.tile([128, TILE_N], f32)
                        for tap in range(9):
                            kh, kw = divmod(tap, 3)
                            off = t * TILE_N + kh * WP + kw
                            c0 = h * (9 * 128) + tap * 128
                            nc.tensor.matmul(
                                pt[:, :n],
                                lhsT=wsb[:, c0 : c0 + 128],
                                rhs=xp[:, off : off + n],
                                start=(tap == 0),
                                stop=(tap == 8),
                            )
                        ot = opool.tile([128, TILE_N], f32)
                        nc.vector.tensor_scalar_add(
                            ot[:, :n], pt[:, :n], bsb[:, h : h + 1]
                        )
                        nc.sync.dma_start(
                            o_d[b, h * 128 : (h + 1) * 128, t * TILE_N : t * TILE_N + n],
                            ot[:, :n],
                        )
    nc.compile()
    return nc


def _get_nc():
    global _nc_cache
    if _nc_cache is None:
        _nc_cache = _build()
    return _nc_cache


def _prep_inputs(x, weights, bias):
    x = np.ascontiguousarray(np.asarray(x, dtype=np.float32))
    weights = np.ascontiguousarray(np.asarray(weights, dtype=np.float32))
    bias = np.ascontiguousarray(np.asarray(bias, dtype=np.float32))

    xp = np.pad(x, ((0, 0), (0, 0), (1, 1), (1, 1))).reshape(B, CIN, NPIX)
    # [Cout,Cin,3,3] -> [Cin, (half kh kw co)] so each Cout-half's taps are
    # one contiguous 1152-col block and each lhsT tap slice is contiguous.
    wT = np.ascontiguousarray(
        weights.reshape(2, 128, CIN, 3, 3).transpose(2, 0, 3, 4, 1)
    ).reshape(CIN, 9 * COUT)
    b2 = np.ascontiguousarray(bias.reshape(2, 128).T)  # b2[p, h] = bias[h*128+p]

    return [
        {
            "xp": np.ascontiguousarray(xp[i * BLOC : (i + 1) * BLOC]),
            "wT": wT,
            "bias2": b2,
        }
        for i in range(NCORES)
    ]


def _run(inputs, trace=False):
    in_maps = _prep_inputs(inputs["x"], inputs["weights"], inputs["bias"])
    res = run_bass_kernel_spmd(
        _get_nc(), in_maps, core_ids=list(range(NCORES)), trace=trace
    )
    out = np.concatenate([r["out"] for r in res.results], axis=0)
    out = out.reshape(B, COUT, H, WP)[:, :, :, :W]
    return np.ascontiguousarray(out), res


def kernel(x, weights, bias):
    out, _ = _run({"x": x, "weights": weights, "bias": bias})
    return out
